# revision 6
# baseline (speedup 1.0000x reference)
"""GPTQ 4-bit quantized linear: out = x @ dequant(qweight, qzeros, scales, g_idx) + bias.

Full shapes: x [8192, 4096] fp16, qweight [512, 4096] int32 (8x 4-bit packed
along K), qzeros [32, 512] int32, scales [32, 4096] fp16, g_idx [4096] int32
(k // 128), bias [4096] fp16.  Output [8192, 4096] fp16.

Strategy: 2 (M) x 4 (N) grid over 8 NeuronCores.  Per core: M=4096, N=1024,
K=4096.  Host does layout only (transpose x, unpack the tiny qzeros, expand
zeros/scales rows to partition layout).  Device dequantizes the weight shard
with DVE (fused shift+mask tensor_scalar per nibble plane j, then subtract
zero / multiply scale) and runs the matmul on the PE accumulating 32 k-planes
(a-chunk x j-plane) per PSUM tile; x is read with stride-8 k-rows so each
nibble plane lines up with its x rows without any on-device transpose.
"""

import os
import sys

import numpy as np

for _p in ("/opt/trn_rl_repo",):
    if _p not in sys.path and os.path.isdir(_p):
        sys.path.insert(0, _p)

import concourse.bass as bass
import concourse.mybir as mybir
import concourse.tile as tile
from concourse import bacc
from concourse.bass_utils import run_bass_kernel_spmd

dt = mybir.dt

P = 128          # partitions
JP = 8           # 4-bit values per int32
KA = P * JP      # k's covered by one a-chunk (1024)
NPS = 512        # psum free width
GROUP = 128      # quant group size == k-chunk size


def build_program(K, M, N):
    """One-core SPMD program: out[M,N] = xt.T @ W + bias with W dequantized
    on the fly.  xt is x-transposed [K, M]."""
    A = K // KA          # a-chunks
    NB = N // NPS        # psum column blocks
    MS = M // NPS        # m superblocks (DMA granularity)
    assert K % KA == 0 and N % NPS == 0 and M % NPS == 0

    nc = bacc.Bacc("TRN2", target_bir_lowering=False)

    xt = nc.dram_tensor("xt", [K, M], dt.float16, kind="ExternalInput")
    qw = nc.dram_tensor("qw", [K // JP, N], dt.int32, kind="ExternalInput")
    zp = nc.dram_tensor("zp", [K // JP, N], dt.int32, kind="ExternalInput")
    sc = nc.dram_tensor("sc", [K // JP, N], dt.float16, kind="ExternalInput")
    bs = nc.dram_tensor("bs", [P, N], dt.float16, kind="ExternalInput")
    out = nc.dram_tensor("out", [M, N], dt.float16, kind="ExternalOutput")

    # k = KA*a + 8*p + j  (p = partition, j = nibble plane)
    xt_r = xt.rearrange("(a p j) m -> a j p m", p=P, j=JP)

    from contextlib import ExitStack

    with tile.TileContext(nc) as tc, ExitStack() as ctx:
        const = ctx.enter_context(tc.tile_pool(name="const", bufs=1))
        qpool = ctx.enter_context(tc.tile_pool(name="qpool", bufs=2))
        zpool = ctx.enter_context(tc.tile_pool(name="zpool", bufs=2))
        spool = ctx.enter_context(tc.tile_pool(name="spool", bufs=2))
        tpool = ctx.enter_context(tc.tile_pool(name="tpool", bufs=3))
        fpool = ctx.enter_context(tc.tile_pool(name="fpool", bufs=3))
        wpool = ctx.enter_context(tc.tile_pool(name="wpool", bufs=2 * A * JP))
        xpool = ctx.enter_context(tc.tile_pool(name="xpool", bufs=int(1.5 * A * JP)))
        opool = ctx.enter_context(tc.tile_pool(name="opool", bufs=4))
        psum = ctx.enter_context(tc.tile_pool(name="psum", bufs=8, space="PSUM"))

        # PE warmup: ~3.8us of dummy matmuls with no DMA dependency, issued
        # during the framework preamble so the HAM clock-gate opens (1.2 ->
        # 2.4 GHz) before the first real matmul.  The HAM activity window is
        # ~3.4us; 9 cold matmuls at ~427ns cover it with margin.
        warm_src = const.tile([P, NPS], dt.float16)
        nc.gpsimd.memset(warm_src[:], 0.0)
        warm_ps = psum.tile([P, NPS], dt.float32, tag="ps")
        NWARM = 9
        for wi in range(NWARM):
            nc.tensor.matmul(
                warm_ps[:], warm_src[:, :P], warm_src[:],
                start=(wi == 0), stop=(wi == NWARM - 1),
            )

        bias_t = None
        wave0_xts = {}
        W = {}
        for nb in range(NB):
            ncol = slice(nb * NPS, (nb + 1) * NPS)
            # --- dequant all (a, j) planes for this n block ---
            # nb0 dequant inputs are on the critical path to the first matmul
            # -> issue them on the Scalar engine's HWDGE queue, which is
            # otherwise idle at kernel start.  This leaves SyncE's in-order
            # issue stream dedicated to x-tile loads, so the first x tile and
            # the first W tile race down parallel DMA queues.  Later blocks
            # are latency-tolerant and go through GpSimd's software queue.
            deq_dma = nc.scalar if nb == 0 else nc.gpsimd
            for a in range(A):
                qw_t = qpool.tile([P, NPS], dt.int32)
                deq_dma.dma_start(qw_t[:], qw[a * P:(a + 1) * P, ncol])
                zp_t = zpool.tile([P, NPS], dt.int32)
                deq_dma.dma_start(zp_t[:], zp[a * P:(a + 1) * P, ncol])
                sc_t = spool.tile([P, NPS], dt.float16)
                deq_dma.dma_start(sc_t[:], sc[a * P:(a + 1) * P, ncol])
                for j in range(JP):
                    if nb == 0:
                        # interleave wave-0 x loads with dequant emission so
                        # the SyncE issue order matches PE consumption order
                        x_t = xpool.tile([P, 2 * NPS], dt.float16, tag="x_t")
                        nc.sync.dma_start(x_t[:], xt_r[a, j, :, 0:2 * NPS])
                        wave0_xts[(a, j)] = x_t
                    ti = tpool.tile([P, NPS], dt.int32)
                    nc.vector.tensor_scalar(
                        ti[:], qw_t[:], 4 * j, 15,
                        op0=mybir.AluOpType.logical_shift_right,
                        op1=mybir.AluOpType.bitwise_and,
                    )
                    tf = fpool.tile([P, NPS], dt.float16)
                    nc.vector.tensor_tensor(
                        tf[:], ti[:], zp_t[:], op=mybir.AluOpType.subtract
                    )
                    w_t = wpool.tile([P, NPS], dt.float16)
                    nc.vector.tensor_tensor(
                        w_t[:], tf[:], sc_t[:], op=mybir.AluOpType.mult
                    )
                    W[(nb, a, j)] = w_t
                if nb == 0 and a == 0:
                    bias_t = const.tile([P, N], dt.float16)
                    nc.scalar.dma_start(bias_t[:], bs[:])

            # --- matmul sweep for this n block ---
            # Waves of 2 m-superblocks = 8 psum tiles, accumulated
            # (a,j)-major so the PE trails the dequant stream with ~no idle
            # on the first wave (each fresh W tile feeds 8 matmuls).
            WAVE_M = 2 * NPS          # 1024 m-cols per wave = 8 psum tiles
            assert M % WAVE_M == 0
            for wave in range(M // WAVE_M):
                mcol = slice(wave * WAVE_M, (wave + 1) * WAVE_M)
                if nb == 0 and wave == 0:
                    xts = [(a, j, wave0_xts[(a, j)])
                           for a in range(A) for j in range(JP)]
                else:
                    xts = []
                    for a in range(A):
                        for j in range(JP):
                            x_t = xpool.tile([P, WAVE_M], dt.float16, tag="x_t")
                            nc.sync.dma_start(x_t[:], xt_r[a, j, :, mcol])
                            xts.append((a, j, x_t))
                last = len(xts) - 1

                def drain(msub, ps, direct=False, out_sync=None, split=False):
                    # Normal waves: ACT drains the psum (frees the bank
                    # without queueing on DVE), DVE adds bias in fp16, and the
                    # store goes out on GpSimd's idle DMA queue so SyncE's
                    # in-order issue stream stays dedicated to x-tile loads.
                    # Final wave (direct=True): one DVE op + fast SyncE store
                    # to shorten the kernel tail.  The very last psum
                    # (split=True) drains as two 256-wide halves down two
                    # HWDGE queues so bias-add and store issue overlap.
                    rows = slice(wave * WAVE_M + msub * P,
                                 wave * WAVE_M + (msub + 1) * P)
                    if split:
                        half = NPS // 2
                        for hi, out_dma in ((0, nc.sync), (1, nc.scalar)):
                            hs = slice(hi * half, (hi + 1) * half)
                            ob = opool.tile([P, half], dt.float16, tag=f"obh{hi}")
                            nc.vector.tensor_tensor(
                                ob[:], ps[:, hs],
                                bias_t[:, nb * NPS + hi * half:
                                       nb * NPS + (hi + 1) * half],
                                op=mybir.AluOpType.add,
                            )
                            out_dma.dma_start(
                                out[rows, nb * NPS + hi * half:
                                    nb * NPS + (hi + 1) * half],
                                ob[:],
                            )
                        return
                    if out_sync is None:
                        out_sync = direct
                    if direct:
                        ob = opool.tile([P, NPS], dt.float16, tag="ob")
                        nc.vector.tensor_tensor(
                            ob[:], ps[:], bias_t[:, ncol], op=mybir.AluOpType.add
                        )
                    else:
                        oc = opool.tile([P, NPS], dt.float16, tag="oc")
                        nc.scalar.copy(oc[:], ps[:])
                        ob = opool.tile([P, NPS], dt.float16, tag="ob")
                        nc.vector.tensor_tensor(
                            ob[:], oc[:], bias_t[:, ncol], op=mybir.AluOpType.add
                        )
                    out_dma = nc.sync if out_sync else nc.gpsimd
                    out_dma.dma_start(
                        out[rows, ncol],
                        ob[:],
                    )

                if nb == NB - 1 and wave == M // WAVE_M - 1:
                    # final wave msub-major: psums drain progressively so the
                    # kernel tail is one psum deep, not eight.
                    n_msub = WAVE_M // P
                    for msub in range(n_msub):
                        ps = psum.tile([P, NPS], dt.float32, tag="ps")
                        for idx, (a, j, x_t) in enumerate(xts):
                            nc.tensor.matmul(
                                ps[:],
                                x_t[:, msub * P:(msub + 1) * P],
                                W[(nb, a, j)][:],
                                start=(idx == 0),
                                stop=(idx == last),
                            )
                        drain(msub, ps, direct=True, split=(msub == n_msub - 1))
                else:
                    # (a,j)-major: each fresh W tile feeds 8 matmuls so the
                    # PE trails the dequant stream with ~no idle (wave 0) and
                    # psum banks all cycle at once (no slot fragmentation).
                    pss = []
                    for msub in range(WAVE_M // P):
                        ps = psum.tile([P, NPS], dt.float32, tag="ps")
                        pss.append((msub, ps))
                    for idx, (a, j, x_t) in enumerate(xts):
                        w_ap = W[(nb, a, j)][:]
                        for (msub, ps) in pss:
                            nc.tensor.matmul(
                                ps[:],
                                x_t[:, msub * P:(msub + 1) * P],
                                w_ap,
                                start=(idx == 0),
                                stop=(idx == last),
                            )
                    for (msub, ps) in pss:
                        drain(msub, ps)
    nc.finalize()
    return nc


def host_prep(x, qweight, qzeros, scales, g_idx, bias, m_split, n_split):
    """Slice + lay out the full inputs into 8 per-core input maps."""
    M_full, K = x.shape
    G, N_full = scales.shape
    M = M_full // m_split
    N = N_full // n_split

    shifts = (np.arange(JP, dtype=np.int32) * 4)
    z = ((qzeros[:, :, None] >> shifts[None, None, :]) & 15).reshape(G, N_full)
    z = z.astype(np.int32) + 1

    # group id per k-chunk of 128 (reference always uses g_idx = k // 128)
    cg = np.asarray(g_idx[::GROUP])
    assert np.array_equal(np.repeat(cg, GROUP), np.asarray(g_idx)), \
        "g_idx must be uniform within 128-wide k chunks"
    z_c = z[cg]                       # [K/128, N_full] int32
    s_c = np.asarray(scales)[cg]      # [K/128, N_full] fp16
    zp_full = np.repeat(z_c, 16, axis=0)   # [K/8, N_full], row 16c+t -> chunk c
    sc_full = np.repeat(s_c, 16, axis=0)

    xT = np.ascontiguousarray(np.asarray(x).T)  # [K, M_full]
    qweight = np.asarray(qweight)
    bias = np.asarray(bias)

    in_maps = []
    for mi in range(m_split):
        for ni in range(n_split):
            nsl = slice(ni * N, (ni + 1) * N)
            in_maps.append({
                "xt": np.ascontiguousarray(xT[:, mi * M:(mi + 1) * M]),
                "qw": np.ascontiguousarray(qweight[:, nsl]),
                "zp": np.ascontiguousarray(zp_full[:, nsl]),
                "sc": np.ascontiguousarray(sc_full[:, nsl]),
                "bs": np.ascontiguousarray(
                    np.broadcast_to(bias[nsl], (P, N))
                ),
            })
    return in_maps, M, N


_PROGRAM_CACHE = {}


def _get_program(K, M, N):
    key = (K, M, N)
    if key not in _PROGRAM_CACHE:
        _PROGRAM_CACHE[key] = build_program(K, M, N)
    return _PROGRAM_CACHE[key]


def kernel(x, qweight, qzeros, scales, g_idx, bias, trace=False, trace_kwargs=None):
    m_split, n_split = 2, 4
    x = np.asarray(x)
    qweight = np.asarray(qweight)
    qzeros = np.asarray(qzeros)
    scales = np.asarray(scales)
    g_idx = np.asarray(g_idx)
    bias = np.asarray(bias)
    M_full, K = x.shape
    N_full = scales.shape[1]
    in_maps, M, N = host_prep(x, qweight, qzeros, scales, g_idx, bias,
                              m_split, n_split)
    nc = _get_program(K, M, N)
    kw = {}
    if trace:
        kw = dict(trace=True, **(trace_kwargs or {}))
    rb = run_bass_kernel_spmd(nc, in_maps, list(range(m_split * n_split)), **kw)
    out = np.empty((M_full, N_full), dtype=np.float16)
    ci = 0
    for mi in range(m_split):
        for ni in range(n_split):
            out[mi * M:(mi + 1) * M, ni * N:(ni + 1) * N] = rb.results[ci]["out"]
            ci += 1
    kernel.last_results = rb
    return out



# revision 11
# speedup vs baseline: 1.0083x; 1.0083x over previous
"""GPTQ 4-bit quantized linear: out = x @ dequant(qweight, qzeros, scales, g_idx) + bias.

Full shapes: x [8192, 4096] fp16, qweight [512, 4096] int32 (8x 4-bit packed
along K), qzeros [32, 512] int32, scales [32, 4096] fp16, g_idx [4096] int32
(k // 128), bias [4096] fp16.  Output [8192, 4096] fp16.

Strategy: 2 (M) x 4 (N) grid over 8 NeuronCores.  Per core: M=4096, N=1024,
K=4096.  Host does layout only (transpose x, unpack the tiny qzeros, expand
zeros/scales rows to partition layout).  Device dequantizes the weight shard
with DVE (fused shift+mask tensor_scalar per nibble plane j, then subtract
zero / multiply scale) and runs the matmul on the PE accumulating 32 k-planes
(a-chunk x j-plane) per PSUM tile; x is read with stride-8 k-rows so each
nibble plane lines up with its x rows without any on-device transpose.
"""

import os
import sys

import numpy as np

for _p in ("/opt/trn_rl_repo",):
    if _p not in sys.path and os.path.isdir(_p):
        sys.path.insert(0, _p)

import concourse.bass as bass
import concourse.mybir as mybir
import concourse.tile as tile
from concourse import bacc
from concourse.bass_utils import run_bass_kernel_spmd

dt = mybir.dt

P = 128          # partitions
JP = 8           # 4-bit values per int32
KA = P * JP      # k's covered by one a-chunk (1024)
NPS = 512        # psum free width
GROUP = 128      # quant group size == k-chunk size


def build_program(K, M, N):
    """One-core SPMD program: out[M,N] = xt.T @ W + bias with W dequantized
    on the fly.  xt is x-transposed [K, M]."""
    A = K // KA          # a-chunks
    NB = N // NPS        # psum column blocks
    MS = M // NPS        # m superblocks (DMA granularity)
    assert K % KA == 0 and N % NPS == 0 and M % NPS == 0

    nc = bacc.Bacc("TRN2", target_bir_lowering=False)

    xt = nc.dram_tensor("xt", [K, M], dt.float16, kind="ExternalInput")
    qw = nc.dram_tensor("qw", [K // JP, N], dt.int32, kind="ExternalInput")
    zp = nc.dram_tensor("zp", [K // JP, N], dt.int32, kind="ExternalInput")
    sc = nc.dram_tensor("sc", [K // JP, N], dt.float16, kind="ExternalInput")
    bs = nc.dram_tensor("bs", [P, N], dt.float16, kind="ExternalInput")
    out = nc.dram_tensor("out", [M, N], dt.float16, kind="ExternalOutput")

    # k = KA*a + 8*p + j  (p = partition, j = nibble plane)
    xt_r = xt.rearrange("(a p j) m -> a j p m", p=P, j=JP)

    from contextlib import ExitStack

    with tile.TileContext(nc) as tc, ExitStack() as ctx:
        const = ctx.enter_context(tc.tile_pool(name="const", bufs=1))
        qpool = ctx.enter_context(tc.tile_pool(name="qpool", bufs=2))
        zpool = ctx.enter_context(tc.tile_pool(name="zpool", bufs=2))
        spool = ctx.enter_context(tc.tile_pool(name="spool", bufs=2))
        tpool = ctx.enter_context(tc.tile_pool(name="tpool", bufs=3))
        fpool = ctx.enter_context(tc.tile_pool(name="fpool", bufs=3))
        wpool = ctx.enter_context(tc.tile_pool(name="wpool", bufs=2 * A * JP))
        xpool = ctx.enter_context(tc.tile_pool(name="xpool", bufs=int(1.5 * A * JP)))
        opool = ctx.enter_context(tc.tile_pool(name="opool", bufs=4))
        psum = ctx.enter_context(tc.tile_pool(name="psum", bufs=8, space="PSUM"))

        # PE warmup: dummy matmuls with no DMA dependency, issued during the
        # framework preamble so the HAM clock-gate opens (1.2 -> 2.4 GHz)
        # before the first real matmul.  The HAM activity window is ~3.4us;
        # 10 cold matmuls at ~427ns end at ~11.2us, right when the first
        # dequantized W tile lands.  The memset goes on DVE, whose queue is
        # free right after the init barrier (GpSimd is still busy with
        # framework loads for ~0.5us).
        warm_src = const.tile([P, NPS], dt.float16)
        nc.vector.memset(warm_src[:], 0.0)
        warm_ps = psum.tile([P, NPS], dt.float32, tag="ps")
        NWARM = 10
        for wi in range(NWARM):
            nc.tensor.matmul(
                warm_ps[:], warm_src[:, :P], warm_src[:],
                start=(wi == 0), stop=(wi == NWARM - 1),
            )

        bias_t = None
        wave0_xts = {}
        W = {}
        for nb in range(NB):
            ncol = slice(nb * NPS, (nb + 1) * NPS)
            # --- dequant all (a, j) planes for this n block ---
            # nb0 dequant inputs are on the critical path to the first matmul
            # -> fast HWDGE issue on SyncE.  Later blocks are latency-tolerant
            # and go through GpSimd's (otherwise idle) DMA queue so they never
            # delay the in-order SyncE stream of x-tile loads mid-sweep.
            deq_dma = nc.sync if nb == 0 else nc.gpsimd
            for a in range(A):
                qw_t = qpool.tile([P, NPS], dt.int32)
                deq_dma.dma_start(qw_t[:], qw[a * P:(a + 1) * P, ncol])
                zp_t = zpool.tile([P, NPS], dt.int32)
                deq_dma.dma_start(zp_t[:], zp[a * P:(a + 1) * P, ncol])
                sc_t = spool.tile([P, NPS], dt.float16)
                deq_dma.dma_start(sc_t[:], sc[a * P:(a + 1) * P, ncol])
                for j in range(JP):
                    if nb == 0:
                        # interleave wave-0 x loads with dequant emission so
                        # the SyncE issue order matches PE consumption order
                        x_t = xpool.tile([P, 2 * NPS], dt.float16, tag="x_t")
                        nc.sync.dma_start(x_t[:], xt_r[a, j, :, 0:2 * NPS])
                        wave0_xts[(a, j)] = x_t
                    ti = tpool.tile([P, NPS], dt.int32)
                    nc.vector.tensor_scalar(
                        ti[:], qw_t[:], 4 * j, 15,
                        op0=mybir.AluOpType.logical_shift_right,
                        op1=mybir.AluOpType.bitwise_and,
                    )
                    tf = fpool.tile([P, NPS], dt.float16)
                    nc.vector.tensor_tensor(
                        tf[:], ti[:], zp_t[:], op=mybir.AluOpType.subtract
                    )
                    w_t = wpool.tile([P, NPS], dt.float16)
                    nc.vector.tensor_tensor(
                        w_t[:], tf[:], sc_t[:], op=mybir.AluOpType.mult
                    )
                    W[(nb, a, j)] = w_t
                if nb == 0 and a == 0:
                    bias_t = const.tile([P, N], dt.float16)
                    nc.sync.dma_start(bias_t[:], bs[:])

            # --- matmul sweep for this n block ---
            # Waves of 2 m-superblocks = 8 psum tiles, accumulated
            # (a,j)-major so the PE trails the dequant stream with ~no idle
            # on the first wave (each fresh W tile feeds 8 matmuls).
            WAVE_M = 2 * NPS          # 1024 m-cols per wave = 8 psum tiles
            assert M % WAVE_M == 0
            for wave in range(M // WAVE_M):
                mcol = slice(wave * WAVE_M, (wave + 1) * WAVE_M)
                if nb == 0 and wave == 0:
                    xts = [(a, j, wave0_xts[(a, j)])
                           for a in range(A) for j in range(JP)]
                else:
                    xts = []
                    for a in range(A):
                        for j in range(JP):
                            x_t = xpool.tile([P, WAVE_M], dt.float16, tag="x_t")
                            nc.sync.dma_start(x_t[:], xt_r[a, j, :, mcol])
                            xts.append((a, j, x_t))
                last = len(xts) - 1

                def drain(msub, ps, direct=False, out_sync=None):
                    # Normal waves: ACT drains the psum (frees the bank
                    # without queueing on DVE), DVE adds bias in fp16, and the
                    # store goes out on GpSimd's idle DMA queue so SyncE's
                    # in-order issue stream stays dedicated to x-tile loads.
                    # Final wave (direct=True): one DVE op + fast SyncE store
                    # to shorten the kernel tail.
                    rows = slice(wave * WAVE_M + msub * P,
                                 wave * WAVE_M + (msub + 1) * P)
                    if out_sync is None:
                        out_sync = direct
                    if direct:
                        ob = opool.tile([P, NPS], dt.float16, tag="ob")
                        nc.vector.tensor_tensor(
                            ob[:], ps[:], bias_t[:, ncol], op=mybir.AluOpType.add
                        )
                    else:
                        oc = opool.tile([P, NPS], dt.float16, tag="oc")
                        nc.scalar.copy(oc[:], ps[:])
                        ob = opool.tile([P, NPS], dt.float16, tag="ob")
                        nc.vector.tensor_tensor(
                            ob[:], oc[:], bias_t[:, ncol], op=mybir.AluOpType.add
                        )
                    out_dma = nc.sync if out_sync else nc.gpsimd
                    out_dma.dma_start(
                        out[rows, ncol],
                        ob[:],
                    )

                if nb == NB - 1 and wave == M // WAVE_M - 1:
                    # final wave msub-major: psums drain progressively so the
                    # kernel tail is one psum deep, not eight.
                    n_msub = WAVE_M // P
                    for msub in range(n_msub):
                        ps = psum.tile([P, NPS], dt.float32, tag="ps")
                        for idx, (a, j, x_t) in enumerate(xts):
                            nc.tensor.matmul(
                                ps[:],
                                x_t[:, msub * P:(msub + 1) * P],
                                W[(nb, a, j)][:],
                                start=(idx == 0),
                                stop=(idx == last),
                            )
                        drain(msub, ps, direct=True)
                else:
                    # (a,j)-major: each fresh W tile feeds 8 matmuls so the
                    # PE trails the dequant stream with ~no idle (wave 0) and
                    # psum banks all cycle at once (no slot fragmentation).
                    pss = []
                    for msub in range(WAVE_M // P):
                        ps = psum.tile([P, NPS], dt.float32, tag="ps")
                        pss.append((msub, ps))
                    for idx, (a, j, x_t) in enumerate(xts):
                        w_ap = W[(nb, a, j)][:]
                        for (msub, ps) in pss:
                            nc.tensor.matmul(
                                ps[:],
                                x_t[:, msub * P:(msub + 1) * P],
                                w_ap,
                                start=(idx == 0),
                                stop=(idx == last),
                            )
                    for (msub, ps) in pss:
                        drain(msub, ps)
    nc.finalize()
    return nc


def host_prep(x, qweight, qzeros, scales, g_idx, bias, m_split, n_split):
    """Slice + lay out the full inputs into 8 per-core input maps."""
    M_full, K = x.shape
    G, N_full = scales.shape
    M = M_full // m_split
    N = N_full // n_split

    shifts = (np.arange(JP, dtype=np.int32) * 4)
    z = ((qzeros[:, :, None] >> shifts[None, None, :]) & 15).reshape(G, N_full)
    z = z.astype(np.int32) + 1

    # group id per k-chunk of 128 (reference always uses g_idx = k // 128)
    cg = np.asarray(g_idx[::GROUP])
    assert np.array_equal(np.repeat(cg, GROUP), np.asarray(g_idx)), \
        "g_idx must be uniform within 128-wide k chunks"
    z_c = z[cg]                       # [K/128, N_full] int32
    s_c = np.asarray(scales)[cg]      # [K/128, N_full] fp16
    zp_full = np.repeat(z_c, 16, axis=0)   # [K/8, N_full], row 16c+t -> chunk c
    sc_full = np.repeat(s_c, 16, axis=0)

    xT = np.ascontiguousarray(np.asarray(x).T)  # [K, M_full]
    qweight = np.asarray(qweight)
    bias = np.asarray(bias)

    in_maps = []
    for mi in range(m_split):
        for ni in range(n_split):
            nsl = slice(ni * N, (ni + 1) * N)
            in_maps.append({
                "xt": np.ascontiguousarray(xT[:, mi * M:(mi + 1) * M]),
                "qw": np.ascontiguousarray(qweight[:, nsl]),
                "zp": np.ascontiguousarray(zp_full[:, nsl]),
                "sc": np.ascontiguousarray(sc_full[:, nsl]),
                "bs": np.ascontiguousarray(
                    np.broadcast_to(bias[nsl], (P, N))
                ),
            })
    return in_maps, M, N


_PROGRAM_CACHE = {}


def _get_program(K, M, N):
    key = (K, M, N)
    if key not in _PROGRAM_CACHE:
        _PROGRAM_CACHE[key] = build_program(K, M, N)
    return _PROGRAM_CACHE[key]


def kernel(x, qweight, qzeros, scales, g_idx, bias, trace=False, trace_kwargs=None):
    m_split, n_split = 2, 4
    x = np.asarray(x)
    qweight = np.asarray(qweight)
    qzeros = np.asarray(qzeros)
    scales = np.asarray(scales)
    g_idx = np.asarray(g_idx)
    bias = np.asarray(bias)
    M_full, K = x.shape
    N_full = scales.shape[1]
    in_maps, M, N = host_prep(x, qweight, qzeros, scales, g_idx, bias,
                              m_split, n_split)
    nc = _get_program(K, M, N)
    kw = {}
    if trace:
        kw = dict(trace=True, **(trace_kwargs or {}))
    rb = run_bass_kernel_spmd(nc, in_maps, list(range(m_split * n_split)), **kw)
    out = np.empty((M_full, N_full), dtype=np.float16)
    ci = 0
    for mi in range(m_split):
        for ni in range(n_split):
            out[mi * M:(mi + 1) * M, ni * N:(ni + 1) * N] = rb.results[ci]["out"]
            ci += 1
    kernel.last_results = rb
    return out

